# revision 10
# baseline (speedup 1.0000x reference)
"""Deformable Conv2d (B=4, C=Co=256, H=W=64, K=3x3, stride=1, pad=1) on 8 trn2 cores.

Strategy (SPMD, core c -> sample b=c//2, parity e=c%2):
  - Decompose deform-conv as: G_k = x^T @ W_k^T per kernel tap (dense GEMM on
    TensorE, position-major output), then bilinear sampling = gather of
    pixel-pair rows of G_k^T + per-position weighted accumulation
    (scalar_tensor_tensor on VectorE with per-partition scalars).
  - Work split: 9 taps x 2 vertical-corner-pairs = 18 "units"; each core takes
    9 units (even: taps {4,0,1,2,3}, odd: {4,5,6,7,8}), computes the matching
    5 G matrices, and produces a partial output. Host sums the two partials
    per sample (linear op) and transposes to [Co, Ho, Wo].
  - Index/bilinear-weight tables are computed on host (tiny: 18*4096 values)
    and passed as inputs; all tensor-scale compute (19.3 GFLOP GEMM + 600 MB
    gather/interp) runs on device.

Self-contained: hardcodes shapes from the problem spec; no sibling imports.
"""
import os
import numpy as np

import concourse.bass as bass
import concourse.bacc as bacc
import concourse.mybir as mybir
import concourse.tile as tile
import concourse.bass_isa as bass_isa
from concourse import library_config
from concourse.bass_utils import run_bass_kernel_spmd
from contextlib import ExitStack

# The 'mlp' GPSIMD library image crashes the exec unit on this runtime when
# running DMAGatherAnt; the identical kernel in 'attnmlp' works. Steer the
# library-load pass to attnmlp by removing the gather ops from mlp's set.
object.__setattr__(
    library_config.mlp, "instructions",
    frozenset(t for t in library_config.mlp.instructions
              if t not in (mybir.InstDMAGatherAnt, bass_isa.InstDMAGather)))

import ml_dtypes

BF16_NP = ml_dtypes.bfloat16

BF = mybir.dt.bfloat16
F32 = mybir.dt.float32
I16 = mybir.dt.int16

B, C, H, W = 4, 256, 64, 64
Co, K = 256, 9
HW = H * W            # 4096
NQG = HW // 128       # 32 position groups
NT = 5                # local taps per core
NU = 9                # units per core
NPAIR = 3             # G gemm tap-pairs: (L0,L1), (L2,L3), (L4,-)

EVEN_TAPS = [4, 0, 1, 2, 3]
ODD_TAPS = [4, 5, 6, 7, 8]
U2T = [0, 1, 1, 2, 2, 3, 3, 4, 4]        # unit -> local tap
U2V_EVEN = [0, 0, 1, 0, 1, 0, 1, 0, 1]   # unit -> vertical corner pair
U2V_ODD = [1, 0, 1, 0, 1, 0, 1, 0, 1]

ACC_DT = BF           # accumulator dtype on device (bf16 fast path)
ACC_NP = BF16_NP


def _unit_table(parity):
    taps = EVEN_TAPS if parity == 0 else ODD_TAPS
    verts = U2V_EVEN if parity == 0 else U2V_ODD
    return [(taps[U2T[u]], verts[u]) for u in range(NU)], taps


def build_nc():
    nc = bacc.Bacc(target_bir_lowering=False)
    xb = nc.declare_dram_parameter("xb", [128, 2, HW], BF, isOutput=False)
    wt = nc.declare_dram_parameter("wt", [128, NPAIR, 2, 512], BF, isOutput=False)
    gidx = nc.declare_dram_parameter("gidx", [128, NU, HW // 16], I16, isOutput=False)
    gwgt = nc.declare_dram_parameter("gwgt", [128, NU, 2, NQG], F32, isOutput=False)
    pout = nc.declare_dram_parameter("pout", [128, NQG, Co], ACC_DT, isOutput=True)

    with ExitStack() as ctx:
        tc = ctx.enter_context(tile.TileContext(nc))
        const = ctx.enter_context(tc.tile_pool(name="const", bufs=1))
        gsb_pool = ctx.enter_context(tc.tile_pool(name="gsb", bufs=2))
        gdram = ctx.enter_context(tc.tile_pool(name="gdram", bufs=1, space="DRAM"))
        psum = ctx.enter_context(tc.tile_pool(name="psum", bufs=2, space="PSUM"))
        gath = ctx.enter_context(tc.tile_pool(name="gath", bufs=3))

        # ---- load inputs ----
        x_sb = const.tile([128, 2, HW], BF)
        nc.sync.dma_start(x_sb[:], xb[:])
        wt_sb = const.tile([128, NPAIR, 2, 512], BF)
        nc.sync.dma_start(wt_sb[:], wt[:])
        gidx_sb = const.tile([128, NU, HW // 16], I16)
        nc.sync.dma_start(gidx_sb[:], gidx[:])
        gwgt_sb = const.tile([128, NU, 2, NQG], F32)
        nc.sync.dma_start(gwgt_sb[:], gwgt[:])

        acc = const.tile([128, NQG, Co], ACC_DT)

        # interleave phase A (G gemm per tap) with phase B (gather+combine):
        # emit tap t's G pipeline, then the units that consume taps <= t.
        g_tiles = [None] * NT   # DRAM tiles, [HW, Co] bf16, pixel-major

        def emit_g(t):
            g_sb = gsb_pool.tile([128, NQG, 256], BF, tag="gsb")
            for qg in range(NQG):
                ps = psum.tile([128, 256], F32, tag="ps")
                for ct in range(2):
                    nc.tensor.matmul(
                        ps[:],
                        x_sb[:, ct, qg * 128:(qg + 1) * 128],
                        wt_sb[:, t // 2, ct, (t % 2) * 256:(t % 2) * 256 + 256],
                        start=(ct == 0),
                        stop=(ct == 1),
                    )
                nc.scalar.activation(
                    g_sb[:, qg, :], ps[:],
                    mybir.ActivationFunctionType.Copy,
                )
            gd = gdram.tile([HW, Co], BF, tag=f"gd{t}")
            g_tiles[t] = gd
            gd_ap = gd[:]
            # DRAM row q = qg*128 + p  <- sbuf [p, qg, :]
            out_ap = bass.AP(
                gd_ap.tensor, gd_ap.offset,
                [[Co, 128], [128 * Co, NQG], [1, Co]],
            )
            nc.sync.dma_start(out_ap, g_sb[:])

        def emit_unit(u):
            gd = g_tiles[U2T[u]]
            gd_ap = gd[:]
            gt = gath.tile([128, NQG, 512], BF, tag="gt")
            in_ap = bass.AP(gd_ap.tensor, gd_ap.offset, [[Co, HW - 1], [1, 512]])
            nc.gpsimd.dma_gather(
                out_ap=gt[:],
                in_ap=in_ap,
                idxs_ap=gidx_sb[:, u, :],
                num_idxs=HW,
                num_idxs_reg=HW,
                elem_size=512,
                elem_step=Co,
                single_packet=False,
            )
            # weighted multiply in place; groups split between DVE (4x mode)
            # and ACT (activation scale) to balance engine load
            for g in range(NQG):
                for c2 in range(2):
                    dst = gt[:, g, c2 * 256:(c2 + 1) * 256]
                    scal = gwgt_sb[:, u, c2, g:g + 1]
                    if (g * 2 + c2) % 2 == 0:
                        nc.vector.tensor_scalar_mul(dst, dst, scal)
                    else:
                        nc.scalar.activation(
                            dst, dst, mybir.ActivationFunctionType.Copy,
                            scale=scal)
            # accumulate (tensor_tensor: 2x DVE mode on bf16)
            gl = gt[:, :, 0:256]
            gr = gt[:, :, 256:512]
            if u == 0:
                nc.vector.tensor_tensor(acc[:], gl, gr, op=mybir.AluOpType.add)
            else:
                nc.vector.tensor_tensor(acc[:], acc[:], gl, op=mybir.AluOpType.add)
                nc.vector.tensor_tensor(acc[:], acc[:], gr, op=mybir.AluOpType.add)

        # tap t is consumed by units [2t-1, 2t] (t>0); unit 0 consumes tap 0
        emit_g(0)
        emit_unit(0)
        for t in range(1, NT):
            emit_g(t)
            emit_unit(2 * t - 1)
            emit_unit(2 * t)

        nc.sync.dma_start(pout[:], acc[:])
    nc.finalize()
    return nc


def _host_idx_weights(offset_b, parity):
    """offset_b [18,64,64] f32 -> lin [NU,HW] int16, wl/wr [NU,HW] f32."""
    units, _ = _unit_table(parity)
    ho = np.arange(H)[:, None]
    wo = np.arange(W)[None, :]
    lin_all = np.zeros((NU, HW), np.int16)
    wl_all = np.zeros((NU, HW), np.float32)
    wr_all = np.zeros((NU, HW), np.float32)
    for u, (gk, v) in enumerate(units):
        off_y = offset_b[2 * gk].astype(np.float64)
        off_x = offset_b[2 * gk + 1].astype(np.float64)
        sy = np.float32(off_y + (ho - 1 + gk // 3)).astype(np.float32)
        sx = np.float32(off_x + (wo - 1 + gk % 3)).astype(np.float32)
        y0 = np.floor(sy)
        x0 = np.floor(sx)
        dy = (sy - y0).astype(np.float32)
        dx = (sx - x0).astype(np.float32)
        y0 = y0.astype(np.int64)
        x0 = x0.astype(np.int64)
        yv = y0 + v
        wy = dy if v == 1 else (np.float32(1.0) - dy)
        vy = (yv >= 0) & (yv < H)
        vl = vy & (x0 >= 0) & (x0 < W)
        vr = vy & (x0 + 1 >= 0) & (x0 + 1 < W)
        wl = (wy * (np.float32(1.0) - dx) * vl).astype(np.float32)
        wr = (wy * dx * vr).astype(np.float32)
        lin = yv * W + x0
        swap_up = lin == -1
        swap_dn = lin == HW - 1
        wl2 = np.where(swap_up, wr, np.where(swap_dn, 0.0, wl))
        wr2 = np.where(swap_up, 0.0, np.where(swap_dn, wl, wr))
        lin2 = lin + swap_up.astype(np.int64) - swap_dn.astype(np.int64)
        lin2 = np.clip(lin2, 0, HW - 2)
        lin_all[u] = lin2.reshape(-1).astype(np.int16)
        wl_all[u] = wl2.reshape(-1)
        wr_all[u] = wr2.reshape(-1)
    return lin_all, wl_all, wr_all


def _core_inputs(x, offset, weight, core):
    b, parity = core // 2, core % 2
    units, taps = _unit_table(parity)

    # xb [128, 2, HW]: xb[p, ct, q] = x[b, ct*128+p, q]
    xf = x[b].reshape(C, HW)
    xb = np.ascontiguousarray(
        xf.reshape(2, 128, HW).transpose(1, 0, 2)).astype(BF16_NP)

    # wt [128, NPAIR, 2, 512]: wt[p, pr, ct, i*256+o] = W[o, ct*128+p, tap L(2pr+i)]
    wt = np.zeros((128, NPAIR, 2, 512), np.float32)
    wk = weight.reshape(Co, C, K)          # [o, c, k]
    for pr in range(NPAIR):
        ntap = 2 if pr < 2 else 1
        for i in range(ntap):
            gk = taps[2 * pr + i]
            wkt = wk[:, :, gk]             # [o, c]
            # -> [p, ct, o]
            wt[:, pr, :, i * 256:(i + 1) * 256] = (
                wkt.T.reshape(2, 128, Co).transpose(1, 0, 2))
    wt = wt.astype(BF16_NP)

    lin, wl, wr = _host_idx_weights(offset[b], parity)
    # gidx [128, NU, HW//16] wrapped-16 + replicated across 8 q7 cores
    gidx = np.zeros((128, NU, HW // 16), np.int16)
    for u in range(NU):
        wrapped = lin[u].reshape(HW // 16, 16).T      # [16, 256]
        gidx[:, u, :] = np.tile(wrapped, (8, 1))
    # gwgt [128, NU, 2, NQG]: [p, u, c2, g] = w_c2[u, g*128+p]
    gwgt = np.stack([wl, wr], axis=1).reshape(NU, 2, NQG, 128)
    gwgt = np.ascontiguousarray(gwgt.transpose(3, 0, 1, 2)).astype(np.float32)

    return {"xb": xb, "wt": wt, "gidx": gidx, "gwgt": gwgt}


_NC_CACHE = {}


def _get_nc():
    if "nc" not in _NC_CACHE:
        _NC_CACHE["nc"] = build_nc()
    return _NC_CACHE["nc"]


def kernel(x, offset, weight):
    x = np.asarray(x, np.float32)
    offset = np.asarray(offset, np.float32)
    weight = np.asarray(weight, np.float32)

    nc = _get_nc()
    core_ids = list(range(8))
    in_maps = [_core_inputs(x, offset, weight, c) for c in core_ids]
    res = run_bass_kernel_spmd(nc, in_maps, core_ids)

    out = np.zeros((B, Co, H, W), np.float32)
    for b in range(B):
        p0 = np.asarray(res.results[2 * b]["pout"]).astype(np.float32)
        p1 = np.asarray(res.results[2 * b + 1]["pout"]).astype(np.float32)
        full = (p0 + p1).transpose(1, 0, 2).reshape(HW, Co)   # [j, o]
        out[b] = full.reshape(H, W, Co).transpose(2, 0, 1)
    return out


# revision 18
# speedup vs baseline: 1.2989x; 1.2989x over previous
"""Deformable Conv2d (B=4, C=Co=256, H=W=64, K=3x3, stride=1, pad=1) on 8 trn2 cores.

Strategy (SPMD, core c -> sample b=c//2, parity e=c%2):
  - Decompose deform-conv as: G_k = x^T @ W_k^T per kernel tap (dense GEMM on
    TensorE, position-major output), then bilinear sampling = gather of
    pixel-pair rows of G_k^T + per-position weighted accumulation
    (scalar_tensor_tensor on VectorE with per-partition scalars).
  - Work split: 9 taps x 2 vertical-corner-pairs = 18 "units"; each core takes
    9 units (even: taps {4,0,1,2,3}, odd: {4,5,6,7,8}), computes the matching
    5 G matrices, and produces a partial output. Host sums the two partials
    per sample (linear op) and transposes to [Co, Ho, Wo].
  - Index/bilinear-weight tables are computed on host (tiny: 18*4096 values)
    and passed as inputs; all tensor-scale compute (19.3 GFLOP GEMM + 600 MB
    gather/interp) runs on device.

Self-contained: hardcodes shapes from the problem spec; no sibling imports.
"""
import os
import numpy as np

import concourse.bass as bass
import concourse.bacc as bacc
import concourse.mybir as mybir
import concourse.tile as tile
import concourse.bass_isa as bass_isa
from concourse import library_config
from concourse.bass_utils import run_bass_kernel_spmd
from contextlib import ExitStack

# The 'mlp' GPSIMD library image crashes the exec unit on this runtime when
# running DMAGatherAnt; the identical kernel in 'attnmlp' works. Steer the
# library-load pass to attnmlp by removing the gather ops from mlp's set.
object.__setattr__(
    library_config.mlp, "instructions",
    frozenset(t for t in library_config.mlp.instructions
              if t not in (mybir.InstDMAGatherAnt, bass_isa.InstDMAGather)))

import ml_dtypes

BF16_NP = ml_dtypes.bfloat16

BF = mybir.dt.bfloat16
F32 = mybir.dt.float32
I16 = mybir.dt.int16

B, C, H, W = 4, 256, 64, 64
Co, K = 256, 9
HW = H * W            # 4096
NQG = HW // 128       # 32 position groups
NT = 5                # local taps per core
NU = 9                # units per core
NPAIR = 3             # G gemm tap-pairs: (L0,L1), (L2,L3), (L4,-)
NPSACC = 6            # PSUM accumulator tiles (2 position-groups each) on PE
ACT_MULT_MOD = 4      # every ACT_MULT_MOD-th weighted-mult goes to ScalarE

EVEN_TAPS = [4, 0, 1, 2, 3]
ODD_TAPS = [4, 5, 6, 7, 8]
U2T = [0, 1, 1, 2, 2, 3, 3, 4, 4]        # unit -> local tap
U2V_EVEN = [0, 0, 1, 0, 1, 0, 1, 0, 1]   # unit -> vertical corner pair
U2V_ODD = [1, 0, 1, 0, 1, 0, 1, 0, 1]

ACC_DT = BF           # accumulator dtype on device (bf16 fast path)
ACC_NP = BF16_NP


def _unit_table(parity):
    taps = EVEN_TAPS if parity == 0 else ODD_TAPS
    verts = U2V_EVEN if parity == 0 else U2V_ODD
    return [(taps[U2T[u]], verts[u]) for u in range(NU)], taps


def build_nc():
    nc = bacc.Bacc(target_bir_lowering=False)
    xb = nc.declare_dram_parameter("xb", [128, 2, HW], BF, isOutput=False)
    wt = nc.declare_dram_parameter("wt", [128, NPAIR, 2, 512], BF, isOutput=False)
    gidx = nc.declare_dram_parameter("gidx", [128, NU, HW // 16], I16, isOutput=False)
    gwgt = nc.declare_dram_parameter("gwgt", [128, NU, 2, NQG], F32, isOutput=False)
    ident = nc.declare_dram_parameter("ident", [128, 128], BF, isOutput=False)
    pout = nc.declare_dram_parameter("pout", [128, NQG, Co], ACC_DT, isOutput=True)

    with ExitStack() as ctx:
        tc = ctx.enter_context(tile.TileContext(nc))
        const = ctx.enter_context(tc.tile_pool(name="const", bufs=1))
        gsb_pool = ctx.enter_context(tc.tile_pool(name="gsb", bufs=2))
        gdram = ctx.enter_context(tc.tile_pool(name="gdram", bufs=1, space="DRAM"))
        psum = ctx.enter_context(tc.tile_pool(name="psum", bufs=2, space="PSUM"))
        gath = ctx.enter_context(tc.tile_pool(name="gath", bufs=3))

        # ---- load inputs ----
        x_sb = const.tile([128, 2, HW], BF)
        nc.sync.dma_start(x_sb[:], xb[:])
        wt_sb = const.tile([128, NPAIR, 2, 512], BF)
        nc.sync.dma_start(wt_sb[:], wt[:])
        gidx_sb = const.tile([128, NU, HW // 16], I16)
        nc.sync.dma_start(gidx_sb[:], gidx[:])
        gwgt_sb = const.tile([128, NU, 2, NQG], F32)
        nc.sync.dma_start(gwgt_sb[:], gwgt[:])
        id_sb = const.tile([128, 128], BF)
        nc.sync.dma_start(id_sb[:], ident[:])

        acc = const.tile([128, NQG, Co], ACC_DT)
        # groups >= NG_DVE accumulate on the PE into PSUM (identity matmul);
        # groups < NG_DVE accumulate on the DVE in SBUF.
        NG_DVE = NQG - 2 * NPSACC
        psacc = [psum.tile([128, 512], F32, tag=f"psacc{i}", bufs=1,
                           name=f"psacc{i}")
                 for i in range(NPSACC)]

        # interleave phase A (G gemm per tap) with phase B (gather+combine):
        # emit tap t's G pipeline, then the units that consume taps <= t.
        g_tiles = [None] * NT   # DRAM tiles, [HW, Co] bf16, pixel-major

        def emit_g(t):
            g_sb = gsb_pool.tile([128, NQG, 256], BF, tag="gsb")
            for qg in range(NQG):
                ps = psum.tile([128, 256], F32, tag="ps")
                for ct in range(2):
                    nc.tensor.matmul(
                        ps[:],
                        x_sb[:, ct, qg * 128:(qg + 1) * 128],
                        wt_sb[:, t // 2, ct, (t % 2) * 256:(t % 2) * 256 + 256],
                        start=(ct == 0),
                        stop=(ct == 1),
                    )
                nc.scalar.activation(
                    g_sb[:, qg, :], ps[:],
                    mybir.ActivationFunctionType.Copy,
                )
            gd = gdram.tile([HW, Co], BF, tag=f"gd{t}")
            g_tiles[t] = gd
            gd_ap = gd[:]
            # DRAM row q = qg*128 + p  <- sbuf [p, qg, :]
            out_ap = bass.AP(
                gd_ap.tensor, gd_ap.offset,
                [[Co, 128], [128 * Co, NQG], [1, Co]],
            )
            nc.sync.dma_start(out_ap, g_sb[:])

        def emit_unit(u):
            gd = g_tiles[U2T[u]]
            gd_ap = gd[:]
            gt = gath.tile([128, NQG, 512], BF, tag="gt")
            in_ap = bass.AP(gd_ap.tensor, gd_ap.offset, [[Co, HW - 1], [1, 512]])
            nc.gpsimd.dma_gather(
                out_ap=gt[:],
                in_ap=in_ap,
                idxs_ap=gidx_sb[:, u, :],
                num_idxs=HW,
                num_idxs_reg=HW,
                elem_size=512,
                elem_step=Co,
                single_packet=False,
            )
            # weighted multiply in place; mostly DVE tensor_scalar (4x mode),
            # a fraction on ACT (activation scale) to balance engine load
            for g in range(NQG):
                for c2 in range(2):
                    dst = gt[:, g, c2 * 256:(c2 + 1) * 256]
                    scal = gwgt_sb[:, u, c2, g:g + 1]
                    if (g * 2 + c2) % ACT_MULT_MOD == ACT_MULT_MOD - 1:
                        nc.scalar.activation(
                            dst, dst, mybir.ActivationFunctionType.Copy,
                            scale=scal)
                    else:
                        nc.vector.tensor_scalar_mul(dst, dst, scal)
            # groups < NG_DVE: accumulate on DVE (tensor_tensor, 2x bf16)
            gl = gt[:, 0:NG_DVE, 0:256]
            gr = gt[:, 0:NG_DVE, 256:512]
            accd = acc[:, 0:NG_DVE, :]
            if u == 0:
                nc.vector.tensor_tensor(accd, gl, gr, op=mybir.AluOpType.add)
            else:
                nc.vector.tensor_tensor(accd, accd, gl, op=mybir.AluOpType.add)
                nc.vector.tensor_tensor(accd, accd, gr, op=mybir.AluOpType.add)
            # groups >= NG_DVE: accumulate on PE via identity matmul into PSUM
            for i in range(NPSACC):
                for gsub in range(2):
                    g = NG_DVE + 2 * i + gsub
                    for c2 in range(2):
                        nc.tensor.matmul(
                            psacc[i][:, gsub * 256:(gsub + 1) * 256],
                            id_sb[:],
                            gt[:, g, c2 * 256:(c2 + 1) * 256],
                            start=(u == 0 and c2 == 0),
                            stop=(u == NU - 1 and c2 == 1),
                            skip_group_check=True,
                        )

        # tap t is consumed by units [2t-1, 2t] (t>0); unit 0 consumes tap 0
        emit_g(0)
        emit_unit(0)
        for t in range(1, NT):
            emit_g(t)
            emit_unit(2 * t - 1)
            emit_unit(2 * t)

        # evict PSUM accumulators into acc, then store the partial output
        for i in range(NPSACC):
            nc.scalar.activation(
                acc[:, NG_DVE + 2 * i:NG_DVE + 2 * i + 2, :], psacc[i][:],
                mybir.ActivationFunctionType.Copy,
            )
        nc.sync.dma_start(pout[:], acc[:])
    nc.finalize()
    return nc


def _host_idx_weights(offset_b, parity):
    """offset_b [18,64,64] f32 -> lin [NU,HW] int16, wl/wr [NU,HW] f32."""
    units, _ = _unit_table(parity)
    ho = np.arange(H)[:, None]
    wo = np.arange(W)[None, :]
    lin_all = np.zeros((NU, HW), np.int16)
    wl_all = np.zeros((NU, HW), np.float32)
    wr_all = np.zeros((NU, HW), np.float32)
    for u, (gk, v) in enumerate(units):
        off_y = offset_b[2 * gk].astype(np.float64)
        off_x = offset_b[2 * gk + 1].astype(np.float64)
        sy = np.float32(off_y + (ho - 1 + gk // 3)).astype(np.float32)
        sx = np.float32(off_x + (wo - 1 + gk % 3)).astype(np.float32)
        y0 = np.floor(sy)
        x0 = np.floor(sx)
        dy = (sy - y0).astype(np.float32)
        dx = (sx - x0).astype(np.float32)
        y0 = y0.astype(np.int64)
        x0 = x0.astype(np.int64)
        yv = y0 + v
        wy = dy if v == 1 else (np.float32(1.0) - dy)
        vy = (yv >= 0) & (yv < H)
        vl = vy & (x0 >= 0) & (x0 < W)
        vr = vy & (x0 + 1 >= 0) & (x0 + 1 < W)
        wl = (wy * (np.float32(1.0) - dx) * vl).astype(np.float32)
        wr = (wy * dx * vr).astype(np.float32)
        lin = yv * W + x0
        swap_up = lin == -1
        swap_dn = lin == HW - 1
        wl2 = np.where(swap_up, wr, np.where(swap_dn, 0.0, wl))
        wr2 = np.where(swap_up, 0.0, np.where(swap_dn, wl, wr))
        lin2 = lin + swap_up.astype(np.int64) - swap_dn.astype(np.int64)
        lin2 = np.clip(lin2, 0, HW - 2)
        lin_all[u] = lin2.reshape(-1).astype(np.int16)
        wl_all[u] = wl2.reshape(-1)
        wr_all[u] = wr2.reshape(-1)
    return lin_all, wl_all, wr_all


def _core_inputs(x, offset, weight, core):
    b, parity = core // 2, core % 2
    units, taps = _unit_table(parity)

    # xb [128, 2, HW]: xb[p, ct, q] = x[b, ct*128+p, q]
    xf = x[b].reshape(C, HW)
    xb = np.ascontiguousarray(
        xf.reshape(2, 128, HW).transpose(1, 0, 2)).astype(BF16_NP)

    # wt [128, NPAIR, 2, 512]: wt[p, pr, ct, i*256+o] = W[o, ct*128+p, tap L(2pr+i)]
    wt = np.zeros((128, NPAIR, 2, 512), np.float32)
    wk = weight.reshape(Co, C, K)          # [o, c, k]
    for pr in range(NPAIR):
        ntap = 2 if pr < 2 else 1
        for i in range(ntap):
            gk = taps[2 * pr + i]
            wkt = wk[:, :, gk]             # [o, c]
            # -> [p, ct, o]
            wt[:, pr, :, i * 256:(i + 1) * 256] = (
                wkt.T.reshape(2, 128, Co).transpose(1, 0, 2))
    wt = wt.astype(BF16_NP)

    lin, wl, wr = _host_idx_weights(offset[b], parity)
    # gidx [128, NU, HW//16] wrapped-16 + replicated across 8 q7 cores
    gidx = np.zeros((128, NU, HW // 16), np.int16)
    for u in range(NU):
        wrapped = lin[u].reshape(HW // 16, 16).T      # [16, 256]
        gidx[:, u, :] = np.tile(wrapped, (8, 1))
    # gwgt [128, NU, 2, NQG]: [p, u, c2, g] = w_c2[u, g*128+p]
    gwgt = np.stack([wl, wr], axis=1).reshape(NU, 2, NQG, 128)
    gwgt = np.ascontiguousarray(gwgt.transpose(3, 0, 1, 2)).astype(np.float32)

    ident = np.eye(128, dtype=BF16_NP)
    return {"xb": xb, "wt": wt, "gidx": gidx, "gwgt": gwgt, "ident": ident}


_NC_CACHE = {}


def _get_nc():
    if "nc" not in _NC_CACHE:
        _NC_CACHE["nc"] = build_nc()
    return _NC_CACHE["nc"]


def kernel(x, offset, weight):
    x = np.asarray(x, np.float32)
    offset = np.asarray(offset, np.float32)
    weight = np.asarray(weight, np.float32)

    nc = _get_nc()
    core_ids = list(range(8))
    in_maps = [_core_inputs(x, offset, weight, c) for c in core_ids]
    res = run_bass_kernel_spmd(nc, in_maps, core_ids)

    out = np.zeros((B, Co, H, W), np.float32)
    for b in range(B):
        p0 = np.asarray(res.results[2 * b]["pout"]).astype(np.float32)
        p1 = np.asarray(res.results[2 * b + 1]["pout"]).astype(np.float32)
        full = (p0 + p1).transpose(1, 0, 2).reshape(HW, Co)   # [j, o]
        out[b] = full.reshape(H, W, Co).transpose(2, 0, 1)
    return out
